# revision 13
# baseline (speedup 1.0000x reference)
"""Trainium2 Bass kernel for the LogicLayer (difflogic) problem.

out[i, o] = c0[o] + ca[o]*a + cb[o]*b + cab[o]*a*b
  with a = x[i, idx_a[o]], b = x[i, idx_b[o]],
  [c0, ca, cb, cab] = softmax(weights[o]) @ GATE_COEFFS.

Strategy: OUTPUT-sharded across 8 cores (1024 outputs/core, all 4096
batch rows); fully DMA-bandwidth-bound, so the a-operand rides as
uint8 fixed point (a ~ (qa+0.5)/256, uniform abs err 1/512) whose
dequantization folds into the per-output coefficients for free:
  t  = qa*(cab/256) + (cb + cab/512)      affine in qa
  r  = qa*(ca/256)                        scale-only in qa
  y  = t*b + r                            b stays fp16 (DVE 2x TT)
  out = y + (c0 + ca/512)                 host epilogue (f32)
HBM traffic per rep/core: 4 MiB qa + 8 MiB b + 8 MiB y ~= 21 MB.

Engines: gathers via SWDGE dma_gather on 4 queues (16 SDMA engines,
8 KiB descriptors); t-affine alternates DVE tensor_scalar / ACT
Identity to balance; r on ACT Copy (float bias); products on DVE 2x.
"""

import numpy as np

BATCH, IN_DIM, OUT_DIM = 4096, 8192, 8192
N_CORES = 8
OPC = OUT_DIM // N_CORES  # 1024 outputs per core
RA = BATCH                # all 4096 rows per core
P = 128
NBLK = OPC // P           # 8 output blocks per core
NIA = 256                 # indices per a-gather (uint8)
NCA = OPC // NIA          # 4 a-chunks
ICA = NIA // 16           # 16 idx cols per a-chunk
NIB = 128                 # indices per b-gather (fp16)
ICB = NIB // 16           # 8 idx cols per b-chunk
T_ON_DVE = (0, 2, 4, 6)   # blocks whose t-affine runs on DVE (rest ACT)

GATE_COEFFS = np.array([
    [0, 0, 0, 0], [0, 0, 0, 1], [0, 1, 0, -1], [0, 1, 0, 0],
    [0, 0, 1, -1], [0, 0, 1, 0], [0, 1, 1, -2], [0, 1, 1, -1],
    [1, -1, -1, 1], [1, -1, -1, 2], [1, 0, -1, 0], [1, 0, -1, 1],
    [1, -1, 0, 0], [1, -1, 0, 1], [1, 0, 0, -1], [1, 0, 0, 0],
], dtype=np.float32)  # [16, 4]

_CACHE = {}


def _build_nc(n_reps=1):
    import concourse.bacc as bacc
    import concourse.mybir as mybir
    from concourse.tile import TileContext

    f32 = mybir.dt.float32
    f16 = mybir.dt.float16
    u8 = mybir.dt.uint8
    i16 = mybir.dt.int16
    Alu = mybir.AluOpType
    Act = mybir.ActivationFunctionType

    nc = bacc.Bacc("TRN2", target_bir_lowering=False, debug=False,
                   num_devices=N_CORES, num_swdge_queues=4)
    xq = nc.dram_tensor("xq", [IN_DIM, RA], u8, kind="ExternalInput").ap()
    xt = nc.dram_tensor("xt", [IN_DIM, RA], f16, kind="ExternalInput").ap()
    idxw = nc.dram_tensor("idxw", [P, NCA * ICA + NBLK * ICB], i16,
                          kind="ExternalInput").ap()
    coef = nc.dram_tensor("coef", [P, 3, NBLK], f32,
                          kind="ExternalInput").ap()
    y = nc.dram_tensor("y", [OPC, RA], f16, kind="ExternalOutput").ap()

    qn = [0]

    def next_q():
        qn[0] = (qn[0] + 1) % 4
        return qn[0]

    with TileContext(nc) as tc:
        with tc.tile_pool(name="const", bufs=1) as cpool, \
             tc.tile_pool(name="qa", bufs=5) as qapool, \
             tc.tile_pool(name="gb", bufs=9) as gbpool, \
             tc.tile_pool(name="tr", bufs=3) as tpool:
            idx_sb = cpool.tile([P, NCA * ICA + NBLK * ICB], i16, tag="idx")
            nc.sync.dma_start(out=idx_sb[:], in_=idxw)
            cf = cpool.tile([P, 3, NBLK], f32, tag="coef")
            nc.sync.dma_start(out=cf[:], in_=coef)

            for rep in range(n_reps):
                qa = None
                for m in range(NBLK):
                    if m % 2 == 0:
                        q = m // 2
                        qa = qapool.tile([P, 2, RA], u8, tag="qa")
                        nc.gpsimd.dma_gather(
                            qa[:], xq, idx_sb[:, q * ICA:(q + 1) * ICA],
                            NIA, NIA, RA, queue_num=next_q())
                    gb = gbpool.tile([P, 1, RA], f16, tag="gb")
                    off = NCA * ICA + m * ICB
                    nc.gpsimd.dma_gather(
                        gb[:], xt, idx_sb[:, off:off + ICB],
                        NIB, NIB, RA, queue_num=next_q())
                    a = qa[:, m % 2, :]
                    b = gb[:, 0, :]
                    t = tpool.tile([P, RA], f16, tag="t")
                    if m in T_ON_DVE:
                        nc.vector.tensor_scalar(
                            t[:], a, cf[:, 0, m:m + 1], cf[:, 1, m:m + 1],
                            Alu.mult, Alu.add)
                    else:
                        nc.scalar.activation(
                            t[:], a, Act.Identity,
                            bias=cf[:, 1, m:m + 1], scale=cf[:, 0, m:m + 1])
                    r = tpool.tile([P, RA], f16, tag="r")
                    nc.scalar.activation(
                        r[:], a, Act.Copy, bias=0.0, scale=cf[:, 2, m:m + 1])
                    nc.vector.tensor_mul(t[:], t[:], b)
                    nc.vector.tensor_add(t[:], t[:], r[:])
                    nc.sync.dma_start(out=y[m * P:(m + 1) * P, :], in_=t[:])
    nc.compile()
    return nc


def _wrap_idx(seq):
    # dma_gather index layout: unwrapped[i] = idxs[i % 16, i // 16],
    # tiled to 128 partitions (replicated across the 8 Q7 cores).
    m = seq.reshape(len(seq) // 16, 16).T
    return np.tile(m, (P // 16, 1))


def _prep_host(x, weights, idx_a, idx_b):
    x = np.asarray(x, dtype=np.float32)
    w = np.asarray(weights, dtype=np.float32)
    e = np.exp(w - w.max(axis=1, keepdims=True))
    sm = e / e.sum(axis=1, keepdims=True)
    C = (sm @ GATE_COEFFS).astype(np.float32)               # [8192, 4]
    c0, ca, cb, cab = C[:, 0], C[:, 1], C[:, 2], C[:, 3]
    cabp = cab / 256.0
    cbp = cb + cab / 512.0
    cap = ca / 256.0
    epi = (c0 + ca / 512.0).astype(np.float32)              # host epilogue

    xT = x.T
    xq = np.ascontiguousarray(
        np.clip(np.floor(xT * 256.0), 0, 255).astype(np.uint8))
    xt = np.ascontiguousarray(xT.astype(np.float16))        # [8192, 4096]
    ia = np.asarray(idx_a).astype(np.int16)
    ib = np.asarray(idx_b).astype(np.int16)

    idxws, cfs, epis = [], [], []
    for c in range(N_CORES):
        lo = c * OPC
        cols = [_wrap_idx(ia[lo + q * NIA:lo + (q + 1) * NIA])
                for q in range(NCA)]
        cols += [_wrap_idx(ib[lo + m * NIB:lo + (m + 1) * NIB])
                 for m in range(NBLK)]
        idxws.append(np.ascontiguousarray(np.concatenate(cols, axis=1)))
        # coef[p, k, m] for output o = lo + m*128 + p; k: cabp, cbp, cap
        cf = np.stack([cabp[lo:lo + OPC], cbp[lo:lo + OPC],
                       cap[lo:lo + OPC]], axis=0)           # [3, 1024]
        cf = cf.reshape(3, NBLK, P).transpose(2, 0, 1)      # [128, 3, 8]
        cfs.append(np.ascontiguousarray(cf))
        epis.append(epi[lo:lo + OPC])
    return xq, xt, idxws, cfs, epis


def _in_maps(x, weights, idx_a, idx_b):
    xq, xt, idxws, cfs, epis = _prep_host(x, weights, idx_a, idx_b)
    _CACHE["epis"] = epis
    return [{"xq": xq, "xt": xt, "idxw": idxws[c], "coef": cfs[c]}
            for c in range(N_CORES)]


def kernel(x, weights, idx_a, idx_b):
    from concourse.bass_utils import run_bass_kernel_spmd

    in_maps = _in_maps(x, weights, idx_a, idx_b)
    epis = _CACHE["epis"]
    if "nc" not in _CACHE:
        _CACHE["nc"] = _build_nc()
    nc = _CACHE["nc"]
    res = run_bass_kernel_spmd(nc, in_maps, list(range(N_CORES)))
    out = np.concatenate(
        [res.results[c]["y"].T.astype(np.float32) + epis[c][None, :]
         for c in range(N_CORES)],
        axis=1)
    return out
